# revision 77
# baseline (speedup 1.0000x reference)
"""Trainium2 Bass kernel for nn_BESNumEigen3qubitModel.

Math reduction (exact):
  vec = rho_vec / ||rho_vec||;  rho = sum_i vec_i G_i + I/8  (Hermitian 8x8, trace 1)
  dm0/dm1 are affine in rho and partial transposes are linear, so every
  eigvalsh in the reference reduces to eigenvalues of 3 Hermitian matrices
  per batch element: rho, pt_a(rho), pt_c(rho):
     beta0 = 1/(1-8 w_min), beta1 = 1/(1-8 w_max),  w = eig(rho)
     loss0 = beta0*(S_k0 - k0/8) + k0/8 ; loss1 = beta1*(T_k1 - k1/8) + k1/8
     loss  = (loss0+loss1)^2 + sum over 4 PPT terms (beta*(ext-1/8)+1/8)^2

Device kernel: batched branchless complex Jacobi in FP16 (validated vs the
f64 reference in numpy with per-op fp16 rounding: max rel err ~4.5e-3 against
the 2e-2 gate), XOR-pair rounds.  Matrices are stored column-major (float
f = 8*col + row, re half then im half) so column updates are contiguous and
hit the DVE/Pool 2-byte 2x ALU modes; per-round rotation coefficients are
materialised into packed fp16 tiles for the same reason.  Two streams (rho:
32 matrices/partition, pt_a+pt_c: 64) are emitted interleaved so each
stream's serial angle-chain latency hides under the other stream's update
bulk; the 4 pair angles per round are computed 4-wide in fp32 via strided
bit-dim views.  Schedule: rho = 3 full sweeps + round 1 of a 4th + 1
angle-only round; PTs = 2 full sweeps + rounds 1-2 of a 3rd + 1 angle-only
round.  Update passes skip the pivot rows (analytic diagonal patch + pivot
annihilation close each round).
"""

import numpy as np

D = 8
BATCH = 32768
NCORES = 8
PER_CORE = BATCH // NCORES       # 4096
NTILES = PER_CORE // 128         # 32 batch tiles per core
NM = 3 * NTILES                  # 96 matrices per partition (type-major)
MR = NTILES                      # rho stream matrices per partition
MP = 2 * NTILES                  # pt stream matrices per partition

_f32 = np.float32

# ---------------------------------------------------------------- host prep --

def _gellmann_basis(d):
    mats = []
    for j in range(d):
        for k in range(j + 1, d):
            m = np.zeros((d, d), np.complex128); m[j, k] = 1; m[k, j] = 1
            mats.append(m)
    for j in range(d):
        for k in range(j + 1, d):
            m = np.zeros((d, d), np.complex128); m[j, k] = -1j; m[k, j] = 1j
            mats.append(m)
    for l in range(1, d):
        m = np.zeros((d, d), np.complex128)
        m[np.arange(l), np.arange(l)] = 1
        m[l, l] = -l
        mats.append(np.sqrt(2.0 / (l * (l + 1))) * m)
    return np.stack(mats)


def _build_maps():
    """[64, 384] f32 map: (vec, 1) -> 128 floats each of rho, pt_a, pt_c.

    COLUMN-MAJOR float layout per matrix: f in [0,64) = Re[i,j] at f=8j+i;
    [64,128) = Im[i,j] at 64+8j+i.
    """
    G = _gellmann_basis(D)
    B = np.zeros((64, 128), np.float64)
    for k in range(63):
        B[k, :64] = G[k].real.T.reshape(-1)
        B[k, 64:] = G[k].imag.T.reshape(-1)
    B[63, :64] = (np.eye(D) / D).T.reshape(-1)

    def entry_perm(kind):
        p = np.zeros(64, np.int64)
        for i in range(8):
            for j in range(8):
                if kind == 'a':
                    i2, j2 = (j & 4) | (i & 3), (i & 4) | (j & 3)
                else:
                    i2, j2 = (i & 6) | (j & 1), (j & 6) | (i & 1)
                p[8 * j + i] = 8 * j2 + i2
        return p

    def float_perm(kind):
        e = entry_perm(kind)
        return np.concatenate([e, 64 + e])

    M3 = np.concatenate([B, B[:, float_perm('a')], B[:, float_perm('c')]], axis=1)
    return M3.astype(_f32)


_M3 = None


def _host_prep(rho_vec):
    global _M3
    if _M3 is None:
        _M3 = _build_maps()
    vec = rho_vec.astype(np.float64)
    vec = vec / np.linalg.norm(vec, axis=-1, keepdims=True)
    vec_aug = np.concatenate(
        [vec.astype(_f32), np.ones((vec.shape[0], 1), _f32)], axis=1)
    flat = vec_aug @ _M3                                   # [B, 384]
    arr = flat.reshape(NCORES, NTILES, 128, 3, 128)        # [core, t, p, type, f]
    return [np.ascontiguousarray(
        arr[c].transpose(1, 2, 0, 3).reshape(128, NM * 128)).astype(np.float16)
        for c in range(NCORES)]


# ------------------------------------------------------------ device kernel --

def _round_pairs(r):
    """Pairs of XOR-round r in L-bit-dim order; L = {j : bit_h(r)=0}, h=msb(r)."""
    h = 1 << (r.bit_length() - 1)
    bits = [b for b in (4, 2, 1) if b != h]
    L = [v1 + v0 for v1 in (0, bits[0]) for v0 in (0, bits[1])]
    return h, bits, [(p, p ^ r) for p in L]


# Batcher odd-even mergesort for 8, grouped into 6 stages of disjoint strided
# comparators: (offset-of-a-set, a-dims, partner-delta) in diag-stride-9 units.
_SORT_STAGES = [
    (0,  [[18, 4]],          9),    # (0,1)(2,3)(4,5)(6,7)
    (0,  [[36, 2], [9, 2]],  18),   # (0,2)(1,3)(4,6)(5,7)
    (9,  [[36, 2]],          9),    # (1,2)(5,6)
    (0,  [[9, 4]],           36),   # (0,4)(1,5)(2,6)(3,7)
    (18, [[9, 2]],           18),   # (2,4)(3,5)
    (9,  [[18, 3]],          9),    # (1,2)(3,4)(5,6)
]


def _build_program(k0, k1, dbg=None):
    import concourse.bass as bass
    import concourse.bacc as bacc
    import concourse.mybir as mybir
    from concourse.tile import TileContext
    from contextlib import ExitStack

    f32 = mybir.dt.float32
    f16 = mybir.dt.float16
    ALU = mybir.AluOpType
    ACT = mybir.ActivationFunctionType

    nc = bacc.Bacc("TRN2")
    mats_d = nc.dram_tensor("mats", [128, NM * 128], f16, kind="ExternalInput")
    out_d = nc.dram_tensor("out", [128, NTILES], f32, kind="ExternalOutput")
    if dbg is not None:
        dbg_d = nc.dram_tensor("dbg", [128, NM * 128], f16, kind="ExternalOutput")

    with ExitStack() as ctx:
        tc = ctx.enter_context(TileContext(nc))
        main = ctx.enter_context(tc.tile_pool(name="main", bufs=1))
        ang = ctx.enter_context(tc.tile_pool(name="ang", bufs=1))
        upd = ctx.enter_context(tc.tile_pool(name="upd", bufs=2))

        AR = main.tile([128, MR, 128], f16, name="AR")
        AT = main.tile([128, MP, 128], f16, name="AT")
        nc.sync.dma_start(out=AR[:], in_=mats_d[:, 0:MR * 128])
        nc.sync.dma_start(out=AT[:], in_=mats_d[:, MR * 128:NM * 128])

        eps30 = main.tile([128, 1], f32, name="eps30")
        nc.vector.memset(eps30[:], 1e-30)
        eps35 = main.tile([128, 1], f32, name="eps35")
        nc.vector.memset(eps35[:], 1e-35)

        def V(base, off, *dims):
            return bass.AP(tensor=base.tensor, offset=base.offset + off,
                           ap=[list(base.ap[0])] + [list(d) for d in dims])

        def vtt(out, a, b, op):
            nc.vector.tensor_tensor(out, a, b, op)

        def vstt(out, a, s, b, op0, op1):
            nc.vector.scalar_tensor_tensor(out, a, s, b, op0, op1)

        NSTAGES = 10

        def round_ctx(A, M, r, do_update, tag):
            """Per-round context: strided views + angle tiles for one stream.

            Column-major entry (i,j) lives at float 8j+i (re) / 64+8j+i (im).
            The pivot X = Re A[p,q] (p in L, q = p^r) sits at 8q+p = 8(j^r)+j
            over j in L; the conjugate slot (q,p) at 8j+(j^r).
            """
            h, bits, pairs = _round_pairs(r)
            B1, B2 = bits
            Aap = A[:]
            mdim = [128, M]

            def sd(b):           # diag stride for bit b: +9b, flipped if b in r
                return -9 * b if (r & b) else 9 * b

            def sxu(b):          # stride of 8j+(j^r) per bit b
                return 7 * b if (r & b) else 9 * b

            def sxl(b):          # stride of 8(j^r)+j per bit b
                return -7 * b if (r & b) else 9 * b

            c = dict(Aap=Aap, M=M, mdim=mdim, r=r, du=do_update, tag=tag,
                     pairs=pairs)
            c['app4'] = V(Aap, 0, mdim, [9 * B1, 2], [9 * B2, 2])
            c['aqq4'] = V(Aap, 9 * r, mdim, [sd(B1), 2], [sd(B2), 2])
            c['X4'] = V(Aap, 8 * r, mdim, [sxl(B1), 2], [sxl(B2), 2])
            c['Y4'] = V(Aap, 64 + 8 * r, mdim, [sxl(B1), 2], [sxl(B2), 2])
            c['XY4'] = V(Aap, 8 * r, mdim, [64, 2], [sxl(B1), 2],
                         [sxl(B2), 2])
            c['XT4'] = V(Aap, r, mdim, [sxu(B1), 2], [sxu(B2), 2])
            c['YT4'] = V(Aap, 64 + r, mdim, [sxu(B1), 2], [sxu(B2), 2])

            def T4(nm):
                return ang.tile([128, M, 4], f32, tag=f"{tag}{nm}", name=nm)[:]

            for nm in ('sqx', 'sqy', 'b2s', 'g', 'gsq', 's2', 'h', 'ag', 'den',
                       'T', 'sg', 't2', 'cden', 'c4', 'u', 'urb2', 'sr',
                       'tb', 'apc', 'aqc', 'X4c', 'Y4c'):
                c[nm] = T4(nm)
            c['csi'] = ang.tile([128, M, 4, 2], f32, tag=f"{tag}csi",
                                name="csi")[:]
            c['apq8'] = ang.tile([128, M, 8], f32, tag=f"{tag}apq8",
                                 name="apq8")[:]
            c['xy8'] = ang.tile([128, M, 2, 4], f32, tag=f"{tag}xy8",
                                name="xy8")[:]
            ap8 = c['apq8']
            c['apcv'] = bass.AP(tensor=ap8.tensor, offset=ap8.offset,
                                ap=[list(ap8.ap[0]), [8, M], [B1, 2], [B2, 2]])
            c['aqcv'] = bass.AP(tensor=ap8.tensor, offset=ap8.offset + r,
                                ap=[list(ap8.ap[0]), [8, M],
                                    [-B1 if (r & B1) else B1, 2],
                                    [-B2 if (r & B2) else B2, 2]])
            # packed fp16 coefficient tiles: 8 real slots per pair (the
            # 2x mode only requires the innermost dim packed; outer dims may
            # be stride-0), csB carries [si | -si] halves explicitly.
            c['cB'] = upd.tile([128, M, 4, 8], f16, tag=f"{tag}cB",
                               name="cB")[:]
            c['srB'] = upd.tile([128, M, 4, 8], f16, tag=f"{tag}srB",
                                name="srB")[:]
            c['csB'] = upd.tile([128, M, 4, 2, 8], f16, tag=f"{tag}csB",
                                name="csB")[:]
            return c

        def angle_stage(c, k):
            """Emit stage k of the 4-wide batched angle chain (fp32)."""
            du = c['du']
            M = c['M']
            if k == 0:
                Aap, mdim, r = c['Aap'], c['mdim'], c['r']
                nc.scalar.copy(c['apq8'], V(Aap, 0, mdim, [9, 8]))
                nc.scalar.copy(c['xy8'], c['XY4'])
            elif k == 1:
                vtt(c['sqx'], c['xy8'][:, :, 0, :], c['xy8'][:, :, 0, :],
                    ALU.mult)
                vtt(c['sqy'], c['xy8'][:, :, 1, :], c['xy8'][:, :, 1, :],
                    ALU.mult)
                vtt(c['g'], c['apcv'], c['aqcv'], ALU.subtract)
            elif k == 2:
                vtt(c['b2s'], c['sqx'], c['sqy'], ALU.add)       # |apq|^2
                vtt(c['gsq'], c['g'], c['g'], ALU.mult)
            elif k == 3:
                vstt(c['s2'], c['b2s'], 4.0, c['gsq'], ALU.mult, ALU.add)
                nc.scalar.activation(c['h'], c['s2'], ACT.Sqrt, bias=eps30[:])
                nc.scalar.activation(c['ag'], c['g'], ACT.Abs)
            elif k == 4:
                vtt(c['den'], c['ag'], c['h'], ALU.add)
                nc.vector.reciprocal(c['T'], c['den'])           # 1/(|g|+h)
                nc.scalar.sign(c['sg'], c['g'], bias=eps35[:])
            elif k == 5:
                vtt(c['u'], c['T'], c['sg'], ALU.mult)           # Tsg
                vstt(c['tb'], c['u'], 2.0, c['b2s'], ALU.mult, ALU.mult)
                if du:
                    # t^2 = 4 T^2 b2s = 2*tb*Tsg  (sg^2 = 1)
                    vstt(c['t2'], c['tb'], 2.0, c['u'], ALU.mult, ALU.mult)
            elif k == 6 and du:
                nc.scalar.activation(c['cden'], c['t2'], ACT.Sqrt, bias=1.0)
                nc.vector.reciprocal(c['c4'], c['cden'])         # cos
            elif k == 7 and du:
                vstt(c['urb2'], c['u'], 2.0, c['c4'], ALU.mult, ALU.mult)
            elif k == 8 and du:
                vtt(c['sr'], c['urb2'], c['xy8'][:, :, 0, :], ALU.mult)
                vtt(c['csi'][:, :, :, 0], c['urb2'], c['xy8'][:, :, 1, :], ALU.mult)
                nc.scalar.activation(c['csi'][:, :, :, 1], c['csi'][:, :, :, 0],
                                     ACT.Copy, scale=-1.0)       # -si
            elif k == 9 and du:
                # materialise packed fp16 coefficient tiles
                cB, srB, csB = c['cB'], c['srB'], c['csB']
                c4t, sr4, csi = c['c4'], c['sr'], c['csi']
                nc.scalar.copy(
                    srB, bass.AP(tensor=sr4.tensor, offset=sr4.offset,
                                 ap=[list(sr4.ap[0]), [4, M], [1, 4], [0, 8]]))
                nc.scalar.copy(
                    cB, bass.AP(tensor=c4t.tensor, offset=c4t.offset,
                                ap=[list(c4t.ap[0]), [4, M], [1, 4], [0, 8]]))
                nc.vector.tensor_copy(
                    csB, bass.AP(tensor=csi.tensor, offset=csi.offset,
                                 ap=[list(csi.ap[0]), [8, M], [2, 4], [1, 2],
                                     [0, 8]]))

        def upd_pair(c, j):
            """Pair j column update in fp16, packed operands throughout:
            P-side full-width on DVE, Q-side run-split on Pool."""
            if not c['du']:
                return
            Aap, M, mdim, tag = c['Aap'], c['M'], c['mdim'], c['tag']
            p, q = c['pairs'][j]
            runs = [(a, l) for (a, l) in
                    ((0, p), (p + 1, q - p - 1), (q + 1, 7 - q)) if l]

            cB, srB, csB = c['cB'], c['srB'], c['csB']

            def cf(t, a, ln):                # cB/srB pair-j rows a..a+ln
                return bass.AP(tensor=t.tensor, offset=t.offset + 8 * j + a,
                               ap=[list(t.ap[0]), [32, M], [0, 2], [1, ln]])

            def cf16(t):                     # cB/srB pair-j as [M,2,8]
                return bass.AP(tensor=t.tensor, offset=t.offset + 8 * j,
                               ap=[list(t.ap[0]), [32, M], [0, 2], [1, 8]])

            def csf(a, ln):                  # csB pair-j rows a..a+ln
                return bass.AP(tensor=csB.tensor,
                               offset=csB.offset + 16 * j + a,
                               ap=[list(csB.ap[0]), [64, M], [8, 2], [1, ln]])

            def csf16():                     # csB pair-j as [M,2,8]
                return bass.AP(tensor=csB.tensor, offset=csB.offset + 16 * j,
                               ap=[list(csB.ap[0]), [64, M], [8, 2], [1, 8]])

            def colr(cc, a, ln):             # col cc rows a..a+ln, re+im
                return V(Aap, 8 * cc + a, mdim, [64, 2], [1, ln])

            def colsw(cc, a, ln):            # swapped [im;re] halves
                return V(Aap, 64 + 8 * cc + a, mdim, [-64, 2], [1, ln])

            def tmpr(t, a, ln):              # [128,M,16] temp run view
                return bass.AP(tensor=t.tensor, offset=t.offset + a,
                               ap=[list(t.ap[0]), [16, M], [8, 2], [1, ln]])

            P1 = upd.tile([128, M, 16], f16, tag=f"{tag}P1", name="P1")[:]
            P2 = upd.tile([128, M, 16], f16, tag=f"{tag}P2", name="P2")[:]
            Q1 = upd.tile([128, M, 16], f16, tag=f"{tag}Q1", name="Q1")[:]
            Q2 = upd.tile([128, M, 16], f16, tag=f"{tag}Q2", name="Q2")[:]

            Ap16 = V(Aap, 8 * p, mdim, [64, 2], [1, 8])
            Aq16 = V(Aap, 8 * q, mdim, [64, 2], [1, 8])
            Apsw = V(Aap, 64 + 8 * p, mdim, [-64, 2], [1, 8])
            Aqsw = V(Aap, 64 + 8 * q, mdim, [-64, 2], [1, 8])

            # all temps + the P-chain on DVE (fp16 2x): full width for the
            # small stream (per-op setup dominates), pivot-row-skipping runs
            # for the wide stream (element cost dominates).
            if M == MR:
                nc.vector.tensor_tensor(tmpr(P1, 0, 8), cf16(srB), Aq16,
                                        ALU.mult)
                nc.vector.tensor_tensor(tmpr(P2, 0, 8), csf16(), Aqsw,
                                        ALU.mult)
                nc.vector.tensor_tensor(tmpr(Q1, 0, 8), cf16(srB), Ap16,
                                        ALU.mult)
                nc.vector.tensor_tensor(tmpr(Q2, 0, 8), csf16(), Apsw,
                                        ALU.mult)
                nc.vector.tensor_tensor(Ap16, cf16(cB), Ap16, ALU.mult)
                nc.vector.tensor_tensor(Ap16, Ap16, tmpr(P1, 0, 8), ALU.add)
                nc.vector.tensor_tensor(Ap16, Ap16, tmpr(P2, 0, 8), ALU.add)
            else:
                for (a, l) in runs:
                    nc.vector.tensor_tensor(
                        tmpr(P1, a, l), cf(srB, a, l), colr(q, a, l), ALU.mult)
                    nc.vector.tensor_tensor(
                        tmpr(P2, a, l), csf(a, l), colsw(q, a, l), ALU.mult)
                    nc.vector.tensor_tensor(
                        tmpr(Q1, a, l), cf(srB, a, l), colr(p, a, l), ALU.mult)
                    nc.vector.tensor_tensor(
                        tmpr(Q2, a, l), csf(a, l), colsw(p, a, l), ALU.mult)
                for (a, l) in runs:
                    nc.vector.tensor_tensor(
                        colr(p, a, l), cf(cB, a, l), colr(p, a, l), ALU.mult)
                    nc.vector.tensor_tensor(
                        colr(p, a, l), colr(p, a, l), tmpr(P1, a, l), ALU.add)
                    nc.vector.tensor_tensor(
                        colr(p, a, l), colr(p, a, l), tmpr(P2, a, l), ALU.add)
            for (a, l) in runs:
                nc.gpsimd.tensor_tensor(
                    colr(q, a, l), cf(cB, a, l), colr(q, a, l), ALU.mult)
                nc.gpsimd.tensor_tensor(
                    colr(q, a, l), colr(q, a, l), tmpr(Q1, a, l), ALU.subtract)
                nc.gpsimd.tensor_tensor(
                    colr(q, a, l), colr(q, a, l), tmpr(Q2, a, l), ALU.add)
            # Hermitian row restore: rows p,q = conj of new cols p,q.  The
            # next pair only reads its own two columns, so restore those
            # first in a tiny op (shortening the pair-to-pair critical path)
            # and the remaining six columns in bulk afterwards.
            d = q - p
            if j < 3:
                pn, qn = c['pairs'][j + 1]
                dn = qn - pn
                nc.scalar.copy(
                    V(Aap, 8 * pn + p, mdim, [8 * dn, 2], [d, 2]),
                    V(Aap, 8 * p + pn, mdim, [dn, 2], [8 * d, 2]))
                nc.scalar.activation(
                    V(Aap, 64 + 8 * pn + p, mdim, [8 * dn, 2], [d, 2]),
                    V(Aap, 64 + 8 * p + pn, mdim, [dn, 2], [8 * d, 2]),
                    ACT.Copy, scale=-1.0)
                for (ca, lc) in ((0, pn), (pn + 1, qn - pn - 1),
                                 (qn + 1, 7 - qn)):
                    if not lc:
                        continue
                    nc.scalar.copy(
                        V(Aap, 8 * ca + p, mdim, [8, lc], [d, 2]),
                        V(Aap, 8 * p + ca, mdim, [1, lc], [8 * d, 2]))
                    nc.scalar.activation(
                        V(Aap, 64 + 8 * ca + p, mdim, [8, lc], [d, 2]),
                        V(Aap, 64 + 8 * p + ca, mdim, [1, lc], [8 * d, 2]),
                        ACT.Copy, scale=-1.0)
            else:
                nc.scalar.copy(V(Aap, p, mdim, [d, 2], [8, 8]),
                               V(Aap, 8 * p, mdim, [8 * d, 2], [1, 8]))
                nc.scalar.activation(
                    V(Aap, 64 + p, mdim, [d, 2], [8, 8]),
                    V(Aap, 64 + 8 * p, mdim, [8 * d, 2], [1, 8]),
                    ACT.Copy, scale=-1.0)

        def finish_round(c):
            # pivot annihilation + analytic diagonal patch (all 4 pairs)
            if c['du']:
                nc.gpsimd.memset(c['X4'], 0.0)
                nc.gpsimd.memset(c['Y4'], 0.0)
                nc.gpsimd.memset(c['XT4'], 0.0)
                nc.gpsimd.memset(c['YT4'], 0.0)
            nc.vector.tensor_tensor(c['app4'], c['apcv'], c['tb'], ALU.add)
            nc.vector.tensor_tensor(c['aqq4'], c['aqcv'], c['tb'], ALU.subtract)

        # Schedules: (r, do_update) per stream.
        sched_R = [(r, True) for _ in range(3) for r in range(1, 8)]
        sched_R += [(1, True), (2, False)]
        sched_P = [(r, True) for _ in range(2) for r in range(1, 8)]
        sched_P += [(1, True), (2, False)]
        if dbg is not None:
            sched_R = sched_R[:dbg[0]]
            sched_P = sched_P[:dbg[1]]

        # Bresenham-interleave the two streams per slot; angle chains are
        # stage-zipped and pair updates round-robin across the active rounds.
        nRr, nPr = len(sched_R), len(sched_P)
        nslots = max(nRr, nPr)
        seq = []
        iR = iP = 0
        for slot in range(nslots):
            if iR < nRr and iR * nslots <= slot * nRr:
                r, du = sched_R[iR]; iR += 1
                seq.append((AR, MR, r, du, "R"))
            if iP < nPr and iP * nslots <= slot * nPr:
                r, du = sched_P[iP]; iP += 1
                seq.append((AT, MP, r, du, "P"))

        def flush(c):
            for j in range(4):
                upd_pair(c, j)
            finish_round(c)

        prev = None
        for item in seq:
            if prev is not None and prev['tag'] == item[4]:
                flush(prev)
                prev = None
            cur = round_ctx(*item)
            for k in range(NSTAGES):
                angle_stage(cur, k)
                if prev is not None and k >= 6 and k - 6 < 4:
                    upd_pair(prev, k - 6)
            if prev is not None:
                finish_round(prev)
            prev = cur
        if prev is not None:
            flush(prev)

        if dbg is not None:
            nc.sync.dma_start(out=dbg_d[:, 0:MR * 128], in_=AR[:])
            nc.sync.dma_start(out=dbg_d[:, MR * 128:NM * 128], in_=AT[:])

        # ---- rho diagonal sort (stage-batched Batcher network, fp16) ----
        Rap = AR[:]
        mdR = [128, MR]
        for si, (off, adims, dlt) in enumerate(_SORT_STAGES):
            Lv = V(Rap, off, mdR, *adims)
            Uv = V(Rap, off + dlt, mdR, *adims)
            tmin_t = ang.tile([128, MR, 4], f16, tag="Rtmin", name="tmin")
            tb_ = tmin_t[:]
            if len(adims) == 1:
                tm = V(tb_, 0, [4, MR], [1, adims[0][1]])
            else:
                tm = V(tb_, 0, [4, MR], [adims[1][1], adims[0][1]],
                       [1, adims[1][1]])
            nc.vector.tensor_tensor(tm, Lv, Uv, ALU.min)
            nc.vector.tensor_tensor(Uv, Lv, Uv, ALU.max)
            nc.scalar.copy(Lv, tm)

        # ---- widen diagonals to fp32 ----
        wR = main.tile([128, MR, 8], f32, name="wR")
        wP = main.tile([128, MP, 8], f32, name="wP")
        nc.scalar.copy(wR[:], V(Rap, 0, mdR, [9, 8]))
        nc.scalar.copy(wP[:], V(AT[:], 0, [128, MP], [9, 8]))

        # ---- pt_a / pt_c diagonal min/max ----
        mn = main.tile([128, MP], f32, name="mn")[:]
        mx = main.tile([128, MP], f32, name="mx")[:]
        nc.vector.tensor_reduce(mn, wP[:], mybir.AxisListType.X, ALU.min)
        nc.vector.tensor_reduce(mx, wP[:], mybir.AxisListType.X, ALU.max)

        mu_min, nu_min = mn[:, 0:NTILES], mn[:, NTILES:MP]
        mu_max, nu_max = mx[:, 0:NTILES], mx[:, NTILES:MP]

        # ---- loss assembly (fp32) ----
        def L(name):
            return main.tile([128, NTILES], f32, tag=name, name=name)[:]

        w_min = wR[:, :, 0]
        w_max = wR[:, :, 7]
        b0, b1, acc, t1, t2_, t3 = L("b0"), L("b1"), L("acc"), L("t1"), L("t2"), L("t3")

        nc.vector.tensor_scalar(b0, w_min, -8.0, 1.0, ALU.mult, ALU.add)
        nc.vector.reciprocal(b0, b0)
        nc.vector.tensor_scalar(b1, w_max, -8.0, 1.0, ALU.mult, ALU.add)
        nc.vector.reciprocal(b1, b1)

        assert 1 <= k0 <= 8 and 1 <= k1 <= 8
        nc.vector.tensor_reduce(t1, wR[:, :, 0:k0], mybir.AxisListType.X,
                                ALU.add)
        nc.vector.tensor_reduce(t2_, wR[:, :, 8 - k1:8], mybir.AxisListType.X,
                                ALU.add)
        # loss0 = b0*(S_k0 - k0/8) + k0/8 ; loss1 = b1*(T_k1 - k1/8) + k1/8
        nc.vector.tensor_scalar(t1, t1, -k0 / 8.0, None, ALU.add)
        nc.vector.tensor_tensor(t1, t1, b0, ALU.mult)
        nc.vector.tensor_scalar(t2_, t2_, -k1 / 8.0, None, ALU.add)
        nc.vector.tensor_tensor(t2_, t2_, b1, ALU.mult)
        nc.vector.tensor_tensor(t1, t1, t2_, ALU.add)
        nc.vector.tensor_scalar(t1, t1, (k0 + k1) / 8.0, None, ALU.add)
        nc.vector.tensor_tensor(acc, t1, t1, ALU.mult)
        for beta, ext in ((b0, mu_min), (b1, mu_max), (b0, nu_min), (b1, nu_max)):
            nc.vector.tensor_scalar(t3, ext, -0.125, None, ALU.add)
            nc.vector.tensor_tensor(t3, t3, beta, ALU.mult)
            nc.vector.tensor_scalar(t3, t3, 0.125, None, ALU.add)
            nc.vector.tensor_tensor(t3, t3, t3, ALU.mult)
            nc.vector.tensor_tensor(acc, acc, t3, ALU.add)

        nc.sync.dma_start(out=out_d[:, :], in_=acc)

    nc.finalize()
    return nc


_prog_cache = {}


def kernel(rho_vec, rank0, rank1):
    rho_vec = np.asarray(rho_vec, dtype=np.float32)
    k0 = D - int(rank0)
    k1 = D - int(rank1)
    in_arrs = _host_prep(rho_vec)

    from concourse.bass_utils import run_bass_kernel_spmd
    key = (k0, k1)
    if key not in _prog_cache:
        _prog_cache[key] = _build_program(k0, k1)
    nc = _prog_cache[key]
    res = run_bass_kernel_spmd(
        nc, [{"mats": a} for a in in_arrs], core_ids=list(range(NCORES)))
    return np.concatenate(
        [np.asarray(res.results[c]["out"]).T.reshape(-1) for c in range(NCORES)]
    ).astype(np.float32)
